# revision 1
# baseline (speedup 1.0000x reference)
"""Chebyshev (L-inf) pairwise distance matrix on 8 TRN2 NeuronCores.

reference: out[i, j] = max_d |embed1[i, d] - embed2[j, d]|
  embed1: [4096, 32] f32, embed2: [4096, 32] f32, out: [4096, 4096] f32

Sharding: 8 cores = 4 i-quarters x 2 j-halves. Each core computes the
[2048 j, 1024 i] transposed block of the output.

Per-core layout: partition axis = j (16 blocks of 128), free axis = i (1024).
For each j-block and each d, the absdiff |e1[i,d] - e2[j,d]| is computed as
either an ACT activation-Abs (bias = -e2[j,d] per partition) or a DVE
tensor_scalar subtract (4x bf16 perf mode) whose sign bit is then cleared by
a single wide bitwise-and on the uint16 view. The tensor operand is e1's
column d broadcast across the 128 partitions (host-prepped, DMA'd once);
the -e2 bias table is negated on-chip from the e2 load.
The max-reduction over d is an in-place wide max tensor_tensor tree on DVE
(2x bf16 mode). Output is bf16, upcast on host. GPSIMD/PE are unusable here:
walrus rejects TensorTensor/TensorScalar on Pool for core v3, and abs_max
is not encodable at all.
"""

import sys

if "/opt/trn_rl_repo" not in sys.path:
    sys.path.insert(0, "/opt/trn_rl_repo")

from contextlib import ExitStack

import ml_dtypes
import numpy as np

import concourse.bacc as bacc
import concourse.bass as bass
import concourse.tile as tile
from concourse import mybir

BF16 = ml_dtypes.bfloat16

N = 4096          # rows of embed1 (= rows of embed2)
D = 32            # feature dim
N_CORES = 8
N_IQ = 4          # i split (embed1 rows)
N_JH = 2          # j split (embed2 rows)
I_PER = N // N_IQ       # 1024 per core
J_PER = N // N_JH       # 2048 per core
JB = J_PER // 128       # 16 j-blocks per core
U = I_PER               # free-dim elements per d-slot

# d ownership for the absdiff stage: ACT does d[0:23], DVE d[23:32].
N_ACT = 23
N_DVE = 9
assert N_ACT + N_DVE == D
# e1r_act arrives in chunks so the first ACT ops don't wait on the full 6 MB;
# tiny first chunk => ACT starts after ~0.5 MB of DMA instead of 2 MB.
ACT_CHUNKS = (2, 7, 7, 7)
assert sum(ACT_CHUNKS) == N_ACT

_nc_cache = None


def _build_nc():
    nc = bacc.Bacc(
        trn_type="TRN2",
        target_bir_lowering=False,
        debug=False,
        num_devices=N_CORES,
    )

    dt_bf16 = mybir.dt.bfloat16
    dt_u16 = mybir.dt.uint16
    dt_f32 = mybir.dt.float32

    # e1 slab transposed to d-major and broadcast across 128 partitions
    # (host side), split by absdiff owner.
    e1r_act = nc.declare_dram_parameter("e1r_act", [128, N_ACT * U], dt_bf16, isOutput=False)
    e1r_dve = nc.declare_dram_parameter("e1r_dve", [128, N_DVE * U], dt_bf16, isOutput=False)
    # e2 j-half slab [J_PER, 32] f32 (negated on-chip for the ACT bias).
    e2b = nc.declare_dram_parameter("e2b", [J_PER, D], dt_f32, isOutput=False)
    out = nc.declare_dram_parameter("out", [J_PER, I_PER], dt_bf16, isOutput=True)

    vmax = mybir.AluOpType.max
    sub = mybir.AluOpType.subtract
    band = mybir.AluOpType.bitwise_and

    with tile.TileContext(nc) as tc, ExitStack() as ctx:
        p_e1 = ctx.enter_context(tc.tile_pool(name="e1", bufs=1))
        p_e2 = ctx.enter_context(tc.tile_pool(name="e2", bufs=1))
        p_act = ctx.enter_context(tc.tile_pool(name="ract", bufs=2))
        p_dve = ctx.enter_context(tc.tile_pool(name="rdve", bufs=1))
        p_out = ctx.enter_context(tc.tile_pool(name="out", bufs=2))

        # --- one-time loads, smallest-first so both engines start early ---
        t_e2 = p_e2.tile([128, JB * D], dt_f32, tag="e2")
        t_e2n = p_e2.tile([128, JB * D], dt_f32, tag="e2n")
        e2_src = e2b[:, :].rearrange("(jb p) d -> p jb d", p=128)
        nc.sync.dma_start(t_e2[:].rearrange("p (jb d) -> p jb d", d=D), e2_src)
        # ACT bias wants -e2; negate on-chip (tiny op) instead of a 2nd DMA
        nc.vector.tensor_scalar(t_e2n[:], t_e2[:], -1.0, None,
                                op0=mybir.AluOpType.mult)

        # first ACT chunk (2 slabs) lands fast so ACT starts ~4us in; the whole
        # DVE region (9 slabs) next; remaining ACT chunks stream in behind
        t_e1a_chunks = []
        off_a = ACT_CHUNKS[0]
        t0 = p_e1.tile([128, ACT_CHUNKS[0] * U], dt_bf16, tag="e1a0")
        nc.sync.dma_start(t0[:], e1r_act[:, :off_a * U])
        t_e1a_chunks.append((0, ACT_CHUNKS[0], t0))
        t_e1d = p_e1.tile([128, N_DVE * U], dt_bf16, tag="e1d")
        nc.sync.dma_start(t_e1d[:], e1r_dve[:, :])
        for ci, csz in enumerate(ACT_CHUNKS[1:], 1):
            t = p_e1.tile([128, csz * U], dt_bf16, tag=f"e1a{ci}")
            nc.sync.dma_start(t[:], e1r_act[:, off_a * U:(off_a + csz) * U])
            t_e1a_chunks.append((off_a, csz, t))
            off_a += csz

        def emit_block(jb, i_lo, w, seq_ract=False):
            """absdiff + reduce + store for j-block jb, i-range [i_lo, i_lo+w)."""
            r_a = p_act.tile([128, N_ACT * w], dt_bf16, tag="ract")
            r_d = p_dve.tile([128, N_DVE * w], dt_bf16, tag="rdve")

            # --- absdiff stage ---
            # DVE's independent work first (keeps DVE busy while ACT runs)
            for k in range(N_DVE):
                d = N_ACT + k
                # raw diff; abs happens in the wide sign-clear below
                nc.vector.tensor_scalar(
                    r_d[:, k * w:(k + 1) * w],
                    t_e1d[:, k * U + i_lo:k * U + i_lo + w],
                    t_e2[:, jb * D + d: jb * D + d + 1],
                    None,
                    op0=sub,
                )
            # clear bf16 sign bits of the whole DVE region in one wide op
            r_d_u16 = r_d[:].bitcast(dt_u16)
            nc.vector.tensor_scalar(r_d_u16, r_d_u16, 0x7FFF, None, op0=band)
            for off, csz, t in t_e1a_chunks:
                for kk in range(csz):
                    k = off + kk
                    d = k
                    # out = Abs(in * 1.0 + (-e2col))
                    nc.scalar.activation(
                        r_a[:, k * w:(k + 1) * w],
                        t[:, kk * U + i_lo:kk * U + i_lo + w],
                        mybir.ActivationFunctionType.Abs,
                        bias=t_e2n[:, jb * D + d: jb * D + d + 1],
                        scale=1.0,
                    )

            # --- reduction: in-place wide max trees (DVE) ---
            # DVE region: 9 slots -> 4 -> 2 -> 1 (+ ragged 9th)
            nc.vector.tensor_tensor(r_d[:, :4 * w], r_d[:, :4 * w], r_d[:, 4 * w:8 * w], op=vmax)
            nc.vector.tensor_tensor(r_d[:, :2 * w], r_d[:, :2 * w], r_d[:, 2 * w:4 * w], op=vmax)
            nc.vector.tensor_tensor(r_d[:, :w], r_d[:, :w], r_d[:, w:2 * w], op=vmax)
            nc.vector.tensor_tensor(r_d[:, :w], r_d[:, :w], r_d[:, 8 * w:9 * w], op=vmax)
            if seq_ract:
                # last block: narrow sequential accumulation — each max op
                # chases the matching ACT absdiff, so after ACT's final slab
                # only ~1 op remains (shrinks the kernel-tail bubble)
                for k in range(1, N_ACT):
                    nc.vector.tensor_tensor(r_a[:, :w], r_a[:, :w],
                                            r_a[:, k * w:(k + 1) * w], op=vmax)
            else:
                # ACT region: 23 slots -> 8(+7) -> 4 -> 2 -> 1
                nc.vector.tensor_tensor(r_a[:, :8 * w], r_a[:, :8 * w], r_a[:, 8 * w:16 * w], op=vmax)
                nc.vector.tensor_tensor(r_a[:, :7 * w], r_a[:, :7 * w], r_a[:, 16 * w:23 * w], op=vmax)
                nc.vector.tensor_tensor(r_a[:, :4 * w], r_a[:, :4 * w], r_a[:, 4 * w:8 * w], op=vmax)
                nc.vector.tensor_tensor(r_a[:, :2 * w], r_a[:, :2 * w], r_a[:, 2 * w:4 * w], op=vmax)
                nc.vector.tensor_tensor(r_a[:, :w], r_a[:, :w], r_a[:, w:2 * w], op=vmax)

            # --- final merge + store ---
            t_out = p_out.tile([128, w], dt_bf16, tag="out")
            nc.vector.tensor_tensor(t_out[:], r_a[:, :w], r_d[:, :w], op=vmax)

            nc.sync.dma_start(out[jb * 128:(jb + 1) * 128, i_lo:i_lo + w], t_out[:])

        for jb in range(JB):
            emit_block(jb, 0, U)

    nc.finalize()
    return nc


def _get_nc():
    global _nc_cache
    if _nc_cache is None:
        _nc_cache = _build_nc()
    return _nc_cache


def make_in_maps(embed1: np.ndarray, embed2: np.ndarray):
    """Host-side sharding/prep. Returns in_maps for cores 0..7.

    Core c: iq = c % N_IQ, jh = c // N_IQ.
    """
    embed1 = np.asarray(embed1, dtype=np.float32)
    embed2 = np.asarray(embed2, dtype=np.float32)
    in_maps = []
    for c in range(N_CORES):
        iq, jh = c % N_IQ, c // N_IQ
        e1_slab = embed1[iq * I_PER:(iq + 1) * I_PER, :]      # [1024, 32]
        # d-major flatten, bf16, broadcast to 128 partitions
        flat = np.ascontiguousarray(e1_slab.T).reshape(-1).astype(BF16)  # [32*1024]
        rep = np.ascontiguousarray(np.broadcast_to(flat[None, :], (128, D * I_PER)))
        e2_slab = np.ascontiguousarray(embed2[jh * J_PER:(jh + 1) * J_PER, :])  # [2048, 32]
        in_maps.append({
            "e1r_act": np.ascontiguousarray(rep[:, :N_ACT * U]),
            "e1r_dve": np.ascontiguousarray(rep[:, N_ACT * U:]),
            "e2b": e2_slab,
        })
    return in_maps


def assemble(results) -> np.ndarray:
    """results: list of per-core dicts with 'out' [J_PER, I_PER] bf16."""
    full = np.empty((N, N), dtype=np.float32)
    for c in range(N_CORES):
        iq, jh = c % N_IQ, c // N_IQ
        blk = np.asarray(results[c]["out"]).astype(np.float32)  # [2048, 1024]
        full[iq * I_PER:(iq + 1) * I_PER, jh * J_PER:(jh + 1) * J_PER] = blk.T
    return full


def kernel(embed1: np.ndarray, embed2: np.ndarray) -> np.ndarray:
    from concourse.bass_utils import run_bass_kernel_spmd

    nc = _get_nc()
    in_maps = make_in_maps(np.asarray(embed1), np.asarray(embed2))
    res = run_bass_kernel_spmd(nc, in_maps, core_ids=list(range(N_CORES)))
    return assemble(res.results)


if __name__ == "__main__":
    e1 = np.random.randn(N, D).astype(np.float32)
    e2 = np.random.randn(N, D).astype(np.float32)
    out = kernel(embed1=e1, embed2=e2)
    ref = np.max(np.abs(e1[:, None, :] - e2[None, :, :]), axis=2)
    err = np.abs(out - ref).max() / np.abs(ref).max()
    print("rel err:", err)



# revision 2
# speedup vs baseline: 1.1079x; 1.1079x over previous
"""Chebyshev (L-inf) pairwise distance matrix on 8 TRN2 NeuronCores.

reference: out[i, j] = max_d |embed1[i, d] - embed2[j, d]|
  embed1: [4096, 32] f32, embed2: [4096, 32] f32, out: [4096, 4096] f32

Sharding: 8 cores = 4 i-quarters x 2 j-halves. Each core computes the
[2048 j, 1024 i] transposed block of the output.

Per-core layout: partition axis = j (16 blocks of 128), free axis = i (1024).
d-ownership: ACT computes |e1[i,d] - e2[j,d]| for d in [0,20) as an Abs
activation (bias = -e2[j,d]); d in [20,32) goes to DVE as 6 custom
ABSDIFF2_MAX ops, each consuming TWO dims in one 1x pass:
  out = max(|in0 - c0|, |in1 - c1|)
which replaces 2 absdiffs + 1 merge of the stock-op scheme. The remaining
reduction is a wide tt-max tree (2x bf16). The last block uses narrow
sequential merges that chase ACT's slots to shrink the kernel tail.
Output is bf16, upcast on host.
"""

import sys

if "/opt/trn_rl_repo" not in sys.path:
    sys.path.insert(0, "/opt/trn_rl_repo")

from contextlib import ExitStack

import ml_dtypes
import numpy as np

import concourse.bacc as bacc
import concourse.bass as bass
import concourse.tile as tile
from concourse import mybir

BF16 = ml_dtypes.bfloat16

N = 4096          # rows of embed1 (= rows of embed2)
D = 32            # feature dim
N_CORES = 8
N_IQ = 4          # i split (embed1 rows)
N_JH = 2          # j split (embed2 rows)
I_PER = N // N_IQ       # 1024 per core
J_PER = N // N_JH       # 2048 per core
JB = J_PER // 128       # 16 j-blocks per core
U = I_PER               # free-dim elements per d-slot

# d ownership: ACT does d[0:20] (one Abs per dim), DVE does d[20:32] as
# 6 pair ops (two dims per op).
N_ACT = 20
N_PAIR = (D - N_ACT) // 2
assert N_ACT + 2 * N_PAIR == D
ACT_CHUNKS = (2, 6, 6, 6)
assert sum(ACT_CHUNKS) == N_ACT

_nc_cache = None
_pair_op = None


def _register_pair_op():
    """Register the ABSDIFF2_MAX custom DVE op (runtime append to dve_ops.OPS).

    out = max(|in0 - s0|, |in1 - s1|) per element; s0/s1 are per-partition
    f32 scalars. Runs at 1 elem/cycle (1x) but consumes two feature dims
    and performs their pair-max in a single instruction.
    """
    global _pair_op
    if _pair_op is not None:
        return _pair_op
    from concourse.dve_spec import Spec, Src0, Src1, C0, C1, maxx, lower, _has_src1
    from concourse.dve_uop import DveOpSpec
    from concourse.dve_ops import (
        DveOp, OPS, _SUB_OPCODE_FOR_NAME, _CUSTOM_DVE_ROW_BASE,
    )

    name = "ABSDIFF2_MAX_ANT"
    for op in OPS:
        if op.name == name:
            _pair_op = op
            return op

    body = maxx(maxx(Src0 - C0, C0 - Src0), maxx(Src1 - C1, C1 - Src1))
    ref = lambda in0, in1, s0, s1, imm2: np.maximum(
        np.abs(in0 - s0), np.abs(in1 - s1))
    spec = Spec(body=body, reference=ref)
    row = _CUSTOM_DVE_ROW_BASE + len(OPS)
    assert row < 0x20
    shas = {}
    for ver in ("v3", "v4"):
        s = DveOpSpec(name=name, opcode=row, uops=lower(spec, ver=ver),
                      rd1_en=_has_src1(spec))
        shas[ver] = s.sha(ver)
    op = DveOp(name, spec, subdim=False, uops_sha=shas)
    OPS.append(op)
    _SUB_OPCODE_FOR_NAME[name] = row
    _pair_op = op
    return op


def _build_nc():
    pair_op = _register_pair_op()

    nc = bacc.Bacc(
        trn_type="TRN2",
        target_bir_lowering=False,
        debug=False,
        num_devices=N_CORES,
    )

    dt_bf16 = mybir.dt.bfloat16
    dt_f32 = mybir.dt.float32

    # e1 slab transposed to d-major and broadcast across 128 partitions
    # (host side), split by absdiff owner.
    e1r_act = nc.declare_dram_parameter("e1r_act", [128, N_ACT * U], dt_bf16, isOutput=False)
    e1r_pair = nc.declare_dram_parameter("e1r_pair", [128, 2 * N_PAIR * U], dt_bf16, isOutput=False)
    # e2 j-half slab [J_PER, 32] f32 (negated on-chip for the ACT bias).
    e2b = nc.declare_dram_parameter("e2b", [J_PER, D], dt_f32, isOutput=False)
    out = nc.declare_dram_parameter("out", [J_PER, I_PER], dt_bf16, isOutput=True)

    vmax = mybir.AluOpType.max

    with tile.TileContext(nc) as tc, ExitStack() as ctx:
        p_e1 = ctx.enter_context(tc.tile_pool(name="e1", bufs=1))
        p_e2 = ctx.enter_context(tc.tile_pool(name="e2", bufs=1))
        p_act = ctx.enter_context(tc.tile_pool(name="ract", bufs=2))
        p_pair = ctx.enter_context(tc.tile_pool(name="rpair", bufs=2))
        p_out = ctx.enter_context(tc.tile_pool(name="out", bufs=2))

        # --- one-time loads, smallest-first so both engines start early ---
        t_e2 = p_e2.tile([128, JB * D], dt_f32, tag="e2")
        t_e2n = p_e2.tile([128, JB * D], dt_f32, tag="e2n")
        e2_src = e2b[:, :].rearrange("(jb p) d -> p jb d", p=128)
        nc.sync.dma_start(t_e2[:].rearrange("p (jb d) -> p jb d", d=D), e2_src)
        # ACT bias wants -e2; negate on-chip (tiny op) instead of a 2nd DMA
        nc.vector.tensor_scalar(t_e2n[:], t_e2[:], -1.0, None,
                                op0=mybir.AluOpType.mult)

        # first ACT chunk (2 slabs) lands fast so ACT starts early; the pair
        # region next so DVE can start; remaining ACT chunks stream in behind
        t_e1a_chunks = []
        off_a = ACT_CHUNKS[0]
        t0 = p_e1.tile([128, ACT_CHUNKS[0] * U], dt_bf16, tag="e1a0")
        nc.sync.dma_start(t0[:], e1r_act[:, :off_a * U])
        t_e1a_chunks.append((0, ACT_CHUNKS[0], t0))
        t_e1p = p_e1.tile([128, 2 * N_PAIR * U], dt_bf16, tag="e1p")
        nc.sync.dma_start(t_e1p[:, :6 * U], e1r_pair[:, :6 * U])
        nc.sync.dma_start(t_e1p[:, 6 * U:], e1r_pair[:, 6 * U:])
        for ci, csz in enumerate(ACT_CHUNKS[1:], 1):
            t = p_e1.tile([128, csz * U], dt_bf16, tag=f"e1a{ci}")
            nc.sync.dma_start(t[:], e1r_act[:, off_a * U:(off_a + csz) * U])
            t_e1a_chunks.append((off_a, csz, t))
            off_a += csz

        def emit_block(jb, i_lo, w, chase=False):
            """absdiff + reduce + store for j-block jb, i-range [i_lo, i_lo+w)."""
            r_a = p_act.tile([128, N_ACT * w], dt_bf16, tag="ract")
            r_p = p_pair.tile([128, N_PAIR * w], dt_bf16, tag="rpair")

            # --- DVE pair ops first (independent of ACT) ---
            for k in range(N_PAIR):
                d0 = N_ACT + 2 * k
                d1 = d0 + 1
                nc.vector._custom_dve(
                    pair_op,
                    out=r_p[:, k * w:(k + 1) * w],
                    in0=t_e1p[:, (2 * k) * U + i_lo:(2 * k) * U + i_lo + w],
                    in1=t_e1p[:, (2 * k + 1) * U + i_lo:(2 * k + 1) * U + i_lo + w],
                    s0=t_e2[:, jb * D + d0: jb * D + d0 + 1],
                    s1=t_e2[:, jb * D + d1: jb * D + d1 + 1],
                )

            # --- ACT absdiffs ---
            for off, csz, t in t_e1a_chunks:
                for kk in range(csz):
                    k = off + kk
                    d = k
                    # out = Abs(in * 1.0 + (-e2col))
                    nc.scalar.activation(
                        r_a[:, k * w:(k + 1) * w],
                        t[:, kk * U + i_lo:kk * U + i_lo + w],
                        mybir.ActivationFunctionType.Abs,
                        bias=t_e2n[:, jb * D + d: jb * D + d + 1],
                        scale=1.0,
                    )

            # --- pair-slot tree: 6 -> 3 -> 2 -> 1 ---
            nc.vector.tensor_tensor(r_p[:, :3 * w], r_p[:, :3 * w], r_p[:, 3 * w:6 * w], op=vmax)
            nc.vector.tensor_tensor(r_p[:, :w], r_p[:, :w], r_p[:, w:2 * w], op=vmax)
            nc.vector.tensor_tensor(r_p[:, :w], r_p[:, :w], r_p[:, 2 * w:3 * w], op=vmax)

            if chase:
                # narrow sequential accumulation — each max chases the matching
                # ACT absdiff, so after ACT's final slab only ~1 op remains
                for k in range(1, N_ACT):
                    nc.vector.tensor_tensor(r_a[:, :w], r_a[:, :w],
                                            r_a[:, k * w:(k + 1) * w], op=vmax)
            else:
                # ACT region: 20 slots -> 10 -> 5 -> (2+ragged) -> 1
                nc.vector.tensor_tensor(r_a[:, :10 * w], r_a[:, :10 * w], r_a[:, 10 * w:20 * w], op=vmax)
                nc.vector.tensor_tensor(r_a[:, :5 * w], r_a[:, :5 * w], r_a[:, 5 * w:10 * w], op=vmax)
                nc.vector.tensor_tensor(r_a[:, :2 * w], r_a[:, :2 * w], r_a[:, 2 * w:4 * w], op=vmax)
                nc.vector.tensor_tensor(r_a[:, :w], r_a[:, :w], r_a[:, w:2 * w], op=vmax)
                nc.vector.tensor_tensor(r_a[:, :w], r_a[:, :w], r_a[:, 4 * w:5 * w], op=vmax)

            # --- final merge + store ---
            t_out = p_out.tile([128, w], dt_bf16, tag="out")
            nc.vector.tensor_tensor(t_out[:], r_a[:, :w], r_p[:, :w], op=vmax)

            nc.sync.dma_start(out[jb * 128:(jb + 1) * 128, i_lo:i_lo + w], t_out[:])

        for jb in range(JB):
            emit_block(jb, 0, U, chase=(jb == JB - 1))

    nc.finalize()
    return nc


def _get_nc():
    global _nc_cache
    if _nc_cache is None:
        _nc_cache = _build_nc()
    return _nc_cache


def make_in_maps(embed1: np.ndarray, embed2: np.ndarray):
    """Host-side sharding/prep. Returns in_maps for cores 0..7.

    Core c: iq = c % N_IQ, jh = c // N_IQ.
    """
    embed1 = np.asarray(embed1, dtype=np.float32)
    embed2 = np.asarray(embed2, dtype=np.float32)
    in_maps = []
    for c in range(N_CORES):
        iq, jh = c % N_IQ, c // N_IQ
        e1_slab = embed1[iq * I_PER:(iq + 1) * I_PER, :]      # [1024, 32]
        # d-major flatten, bf16, broadcast to 128 partitions
        flat = np.ascontiguousarray(e1_slab.T).reshape(-1).astype(BF16)  # [32*1024]
        rep = np.ascontiguousarray(np.broadcast_to(flat[None, :], (128, D * I_PER)))
        e2_slab = np.ascontiguousarray(embed2[jh * J_PER:(jh + 1) * J_PER, :])  # [2048, 32]
        in_maps.append({
            "e1r_act": np.ascontiguousarray(rep[:, :N_ACT * U]),
            "e1r_pair": np.ascontiguousarray(rep[:, N_ACT * U:]),
            "e2b": e2_slab,
        })
    return in_maps


def assemble(results) -> np.ndarray:
    """results: list of per-core dicts with 'out' [J_PER, I_PER] bf16."""
    full = np.empty((N, N), dtype=np.float32)
    for c in range(N_CORES):
        iq, jh = c % N_IQ, c // N_IQ
        blk = np.asarray(results[c]["out"]).astype(np.float32)  # [2048, 1024]
        full[iq * I_PER:(iq + 1) * I_PER, jh * J_PER:(jh + 1) * J_PER] = blk.T
    return full


def kernel(embed1: np.ndarray, embed2: np.ndarray) -> np.ndarray:
    from concourse.bass_utils import run_bass_kernel_spmd

    nc = _get_nc()
    in_maps = make_in_maps(np.asarray(embed1), np.asarray(embed2))
    res = run_bass_kernel_spmd(nc, in_maps, core_ids=list(range(N_CORES)))
    return assemble(res.results)


if __name__ == "__main__":
    e1 = np.random.randn(N, D).astype(np.float32)
    e2 = np.random.randn(N, D).astype(np.float32)
    out = kernel(embed1=e1, embed2=e2)
    ref = np.max(np.abs(e1[:, None, :] - e2[None, :, :]), axis=2)
    err = np.abs(out - ref).max() / np.abs(ref).max()
    print("rel err:", err)


# revision 3
# speedup vs baseline: 1.1263x; 1.0166x over previous
"""Chebyshev (L-inf) pairwise distance matrix on 8 TRN2 NeuronCores.

reference: out[i, j] = max_d |embed1[i, d] - embed2[j, d]|
  embed1: [4096, 32] f32, embed2: [4096, 32] f32, out: [4096, 4096] f32

Sharding: 8 cores = 4 i-quarters x 2 j-halves. Each core computes the
[2048 j, 1024 i] transposed block of the output.

Per-core layout: partition axis = j (16 blocks of 128), free axis = i (1024).
d-ownership: ACT computes |e1[i,d] - e2[j,d]| for d in [0,20) as an Abs
activation (bias = -e2[j,d]); d in [20,32) goes to DVE as 6 custom
ABSDIFF2_MAX ops, each consuming TWO dims in one 1x pass:
  out = max(|in0 - c0|, |in1 - c1|)
which replaces 2 absdiffs + 1 merge of the stock-op scheme. The remaining
reduction is a wide tt-max tree (2x bf16). Pair ops are emitted two
j-blocks ahead of the merge stage so DVE never waits on ACT's first
block; e2/-e2 are prepped host-side (contiguous [128, 16*32] f32) so both
engines start within a few us. The last block uses narrow sequential
merges that chase ACT's slots to shrink the kernel tail. Output is bf16,
upcast on host.
"""

import sys

if "/opt/trn_rl_repo" not in sys.path:
    sys.path.insert(0, "/opt/trn_rl_repo")

from contextlib import ExitStack

import ml_dtypes
import numpy as np

import concourse.bacc as bacc
import concourse.bass as bass
import concourse.tile as tile
from concourse import mybir

BF16 = ml_dtypes.bfloat16

N = 4096          # rows of embed1 (= rows of embed2)
D = 32            # feature dim
N_CORES = 8
N_IQ = 4          # i split (embed1 rows)
N_JH = 2          # j split (embed2 rows)
I_PER = N // N_IQ       # 1024 per core
J_PER = N // N_JH       # 2048 per core
JB = J_PER // 128       # 16 j-blocks per core
U = I_PER               # free-dim elements per d-slot

# d ownership: ACT does d[0:20] (one Abs per dim), DVE does d[20:32] as
# 6 pair ops (two dims per op).
N_ACT = 20
N_PAIR = (D - N_ACT) // 2
assert N_ACT + 2 * N_PAIR == D
ACT_CHUNKS = (2, 6, 6, 6)
assert sum(ACT_CHUNKS) == N_ACT
PAIR_CHUNKS = (4, 4, 4)        # e1 pair-slab DMA chunks (slots)
assert sum(PAIR_CHUNKS) == 2 * N_PAIR
PAIR_AHEAD = 2                 # pair ops run this many blocks ahead of merges

_nc_cache = None
_pair_op = None


def _register_pair_op():
    """Register the ABSDIFF2_MAX custom DVE op (runtime append to dve_ops.OPS).

    out = max(|in0 - s0|, |in1 - s1|) per element; s0/s1 are per-partition
    f32 scalars. Runs at 1 elem/cycle (1x) but consumes two feature dims
    and performs their pair-max in a single instruction.
    """
    global _pair_op
    if _pair_op is not None:
        return _pair_op
    from concourse.dve_spec import Spec, Src0, Src1, C0, C1, maxx, lower, _has_src1
    from concourse.dve_uop import DveOpSpec
    from concourse.dve_ops import (
        DveOp, OPS, _SUB_OPCODE_FOR_NAME, _CUSTOM_DVE_ROW_BASE,
    )

    name = "ABSDIFF2_MAX_ANT"
    for op in OPS:
        if op.name == name:
            _pair_op = op
            return op

    body = maxx(maxx(Src0 - C0, C0 - Src0), maxx(Src1 - C1, C1 - Src1))
    ref = lambda in0, in1, s0, s1, imm2: np.maximum(
        np.abs(in0 - s0), np.abs(in1 - s1))
    spec = Spec(body=body, reference=ref)
    row = _CUSTOM_DVE_ROW_BASE + len(OPS)
    assert row < 0x20
    shas = {}
    for ver in ("v3", "v4"):
        s = DveOpSpec(name=name, opcode=row, uops=lower(spec, ver=ver),
                      rd1_en=_has_src1(spec))
        shas[ver] = s.sha(ver)
    op = DveOp(name, spec, subdim=False, uops_sha=shas)
    OPS.append(op)
    _SUB_OPCODE_FOR_NAME[name] = row
    _pair_op = op
    return op


def _build_nc():
    pair_op = _register_pair_op()

    nc = bacc.Bacc(
        trn_type="TRN2",
        target_bir_lowering=False,
        debug=False,
        num_devices=N_CORES,
    )

    dt_bf16 = mybir.dt.bfloat16
    dt_f32 = mybir.dt.float32

    # e1 slab transposed to d-major and broadcast across 128 partitions
    # (host side), split by absdiff owner.
    e1r_act = nc.declare_dram_parameter("e1r_act", [128, N_ACT * U], dt_bf16, isOutput=False)
    e1r_pair = nc.declare_dram_parameter("e1r_pair", [128, 2 * N_PAIR * U], dt_bf16, isOutput=False)
    # e2 j-half slab, pre-transposed host-side to [128, JB*D] (p-major) so the
    # load is one contiguous DMA; e2n = -e2 for the ACT bias.
    e2b = nc.declare_dram_parameter("e2b", [128, JB * D], dt_f32, isOutput=False)
    e2nb = nc.declare_dram_parameter("e2nb", [128, JB * D], dt_f32, isOutput=False)
    out = nc.declare_dram_parameter("out", [J_PER, I_PER], dt_bf16, isOutput=True)

    vmax = mybir.AluOpType.max

    with tile.TileContext(nc) as tc, ExitStack() as ctx:
        p_e1 = ctx.enter_context(tc.tile_pool(name="e1", bufs=1))
        p_e2 = ctx.enter_context(tc.tile_pool(name="e2", bufs=1))
        p_act = ctx.enter_context(tc.tile_pool(name="ract", bufs=2))
        p_pair = ctx.enter_context(tc.tile_pool(name="rpair", bufs=PAIR_AHEAD + 1))
        p_out = ctx.enter_context(tc.tile_pool(name="out", bufs=2))

        # --- one-time loads; ordered so DVE (pairs) and ACT start early ---
        t_e2 = p_e2.tile([128, JB * D], dt_f32, tag="e2")
        t_e2n = p_e2.tile([128, JB * D], dt_f32, tag="e2n")
        nc.sync.dma_start(t_e2[:], e2b[:, :])
        nc.sync.dma_start(t_e2n[:], e2nb[:, :])

        t_e1p = p_e1.tile([128, 2 * N_PAIR * U], dt_bf16, tag="e1p")
        t_e1a_chunks = []

        # pair chunks a+b first (DVE's first 4 pair ops), then ACT chunk 0,
        # then the rest interleaved smallest-dependency-first
        po = 0
        pair_dmas = []
        for ci, csz in enumerate(PAIR_CHUNKS):
            pair_dmas.append((po, csz))
            po += csz
        nc.sync.dma_start(t_e1p[:, pair_dmas[0][0] * U:(pair_dmas[0][0] + pair_dmas[0][1]) * U],
                          e1r_pair[:, pair_dmas[0][0] * U:(pair_dmas[0][0] + pair_dmas[0][1]) * U])
        nc.sync.dma_start(t_e1p[:, pair_dmas[1][0] * U:(pair_dmas[1][0] + pair_dmas[1][1]) * U],
                          e1r_pair[:, pair_dmas[1][0] * U:(pair_dmas[1][0] + pair_dmas[1][1]) * U])

        off_a = ACT_CHUNKS[0]
        t0 = p_e1.tile([128, ACT_CHUNKS[0] * U], dt_bf16, tag="e1a0")
        nc.sync.dma_start(t0[:], e1r_act[:, :off_a * U])
        t_e1a_chunks.append((0, ACT_CHUNKS[0], t0))

        nc.sync.dma_start(t_e1p[:, pair_dmas[2][0] * U:(pair_dmas[2][0] + pair_dmas[2][1]) * U],
                          e1r_pair[:, pair_dmas[2][0] * U:(pair_dmas[2][0] + pair_dmas[2][1]) * U])

        for ci, csz in enumerate(ACT_CHUNKS[1:], 1):
            t = p_e1.tile([128, csz * U], dt_bf16, tag=f"e1a{ci}")
            nc.sync.dma_start(t[:], e1r_act[:, off_a * U:(off_a + csz) * U])
            t_e1a_chunks.append((off_a, csz, t))
            off_a += csz

        w = U
        i_lo = 0
        pair_tiles = {}

        def emit_pairs(jb):
            """6 custom pair-absdiff ops for j-block jb (DVE, no ACT dep)."""
            r_p = p_pair.tile([128, N_PAIR * w], dt_bf16, tag="rpair")
            pair_tiles[jb] = r_p
            for k in range(N_PAIR):
                d0 = N_ACT + 2 * k
                d1 = d0 + 1
                nc.vector._custom_dve(
                    pair_op,
                    out=r_p[:, k * w:(k + 1) * w],
                    in0=t_e1p[:, (2 * k) * U + i_lo:(2 * k) * U + i_lo + w],
                    in1=t_e1p[:, (2 * k + 1) * U + i_lo:(2 * k + 1) * U + i_lo + w],
                    s0=t_e2[:, jb * D + d0: jb * D + d0 + 1],
                    s1=t_e2[:, jb * D + d1: jb * D + d1 + 1],
                )

        def emit_act(jb):
            """20 ACT absdiffs for j-block jb."""
            r_a = p_act.tile([128, N_ACT * w], dt_bf16, tag="ract")
            for off, csz, t in t_e1a_chunks:
                for kk in range(csz):
                    k = off + kk
                    d = k
                    # out = Abs(in * 1.0 + (-e2col))
                    nc.scalar.activation(
                        r_a[:, k * w:(k + 1) * w],
                        t[:, kk * U + i_lo:kk * U + i_lo + w],
                        mybir.ActivationFunctionType.Abs,
                        bias=t_e2n[:, jb * D + d: jb * D + d + 1],
                        scale=1.0,
                    )
            return r_a

        def emit_merges(jb, r_a, chase=False):
            r_p = pair_tiles.pop(jb)
            # pair-slot tree: 6 -> 3 -> 2 -> 1
            nc.vector.tensor_tensor(r_p[:, :3 * w], r_p[:, :3 * w], r_p[:, 3 * w:6 * w], op=vmax)
            nc.vector.tensor_tensor(r_p[:, :w], r_p[:, :w], r_p[:, w:2 * w], op=vmax)
            nc.vector.tensor_tensor(r_p[:, :w], r_p[:, :w], r_p[:, 2 * w:3 * w], op=vmax)

            if chase:
                # narrow sequential accumulation — each max chases the matching
                # ACT absdiff, so after ACT's final slab only ~1 op remains
                for k in range(1, N_ACT):
                    nc.vector.tensor_tensor(r_a[:, :w], r_a[:, :w],
                                            r_a[:, k * w:(k + 1) * w], op=vmax)
            else:
                # ACT region: 20 slots -> 10 -> 5 -> (2+ragged) -> 1
                nc.vector.tensor_tensor(r_a[:, :10 * w], r_a[:, :10 * w], r_a[:, 10 * w:20 * w], op=vmax)
                nc.vector.tensor_tensor(r_a[:, :5 * w], r_a[:, :5 * w], r_a[:, 5 * w:10 * w], op=vmax)
                nc.vector.tensor_tensor(r_a[:, :2 * w], r_a[:, :2 * w], r_a[:, 2 * w:4 * w], op=vmax)
                nc.vector.tensor_tensor(r_a[:, :w], r_a[:, :w], r_a[:, w:2 * w], op=vmax)
                nc.vector.tensor_tensor(r_a[:, :w], r_a[:, :w], r_a[:, 4 * w:5 * w], op=vmax)

            # final merge + store
            t_out = p_out.tile([128, w], dt_bf16, tag="out")
            nc.vector.tensor_tensor(t_out[:], r_a[:, :w], r_p[:, :w], op=vmax)
            nc.sync.dma_start(out[jb * 128:(jb + 1) * 128, i_lo:i_lo + w], t_out[:])

        # software pipeline: pairs run PAIR_AHEAD blocks ahead of merges
        for jb in range(PAIR_AHEAD):
            emit_pairs(jb)
        act_tiles = {}
        for jb in range(JB):
            if jb + PAIR_AHEAD < JB:
                emit_pairs(jb + PAIR_AHEAD)
            r_a = emit_act(jb)
            emit_merges(jb, r_a, chase=(jb == JB - 1))

    nc.finalize()
    return nc


def _get_nc():
    global _nc_cache
    if _nc_cache is None:
        _nc_cache = _build_nc()
    return _nc_cache


def make_in_maps(embed1: np.ndarray, embed2: np.ndarray):
    """Host-side sharding/prep. Returns in_maps for cores 0..7.

    Core c: iq = c % N_IQ, jh = c // N_IQ.
    """
    embed1 = np.asarray(embed1, dtype=np.float32)
    embed2 = np.asarray(embed2, dtype=np.float32)
    in_maps = []
    for c in range(N_CORES):
        iq, jh = c % N_IQ, c // N_IQ
        e1_slab = embed1[iq * I_PER:(iq + 1) * I_PER, :]      # [1024, 32]
        # d-major flatten, bf16, broadcast to 128 partitions
        flat = np.ascontiguousarray(e1_slab.T).reshape(-1).astype(BF16)  # [32*1024]
        rep = np.ascontiguousarray(np.broadcast_to(flat[None, :], (128, D * I_PER)))
        e2_slab = embed2[jh * J_PER:(jh + 1) * J_PER, :]  # [2048, 32]
        # [128, JB*D] p-major: e2p[p, jb*D+d] = e2_slab[jb*128+p, d]
        e2p = np.ascontiguousarray(
            e2_slab.reshape(JB, 128, D).transpose(1, 0, 2).reshape(128, JB * D))
        in_maps.append({
            "e1r_act": np.ascontiguousarray(rep[:, :N_ACT * U]),
            "e1r_pair": np.ascontiguousarray(rep[:, N_ACT * U:]),
            "e2b": e2p,
            "e2nb": np.ascontiguousarray(-e2p),
        })
    return in_maps


def assemble(results) -> np.ndarray:
    """results: list of per-core dicts with 'out' [J_PER, I_PER] bf16."""
    full = np.empty((N, N), dtype=np.float32)
    for c in range(N_CORES):
        iq, jh = c % N_IQ, c // N_IQ
        blk = np.asarray(results[c]["out"]).astype(np.float32)  # [2048, 1024]
        full[iq * I_PER:(iq + 1) * I_PER, jh * J_PER:(jh + 1) * J_PER] = blk.T
    return full


def kernel(embed1: np.ndarray, embed2: np.ndarray) -> np.ndarray:
    from concourse.bass_utils import run_bass_kernel_spmd

    nc = _get_nc()
    in_maps = make_in_maps(np.asarray(embed1), np.asarray(embed2))
    res = run_bass_kernel_spmd(nc, in_maps, core_ids=list(range(N_CORES)))
    return assemble(res.results)


if __name__ == "__main__":
    e1 = np.random.randn(N, D).astype(np.float32)
    e2 = np.random.randn(N, D).astype(np.float32)
    out = kernel(embed1=e1, embed2=e2)
    ref = np.max(np.abs(e1[:, None, :] - e2[None, :, :]), axis=2)
    err = np.abs(out - ref).max() / np.abs(ref).max()
    print("rel err:", err)


# revision 8
# speedup vs baseline: 1.1373x; 1.0098x over previous
"""Chebyshev (L-inf) pairwise distance matrix on 8 TRN2 NeuronCores.

reference: out[i, j] = max_d |embed1[i, d] - embed2[j, d]|
  embed1: [4096, 32] f32, embed2: [4096, 32] f32, out: [4096, 4096] f32

Sharding: 8 cores = 4 i-quarters x 2 j-halves. Each core computes the
[2048 j, 1024 i] transposed block of the output.

Per-core layout: partition axis = j (16 blocks of 128), free axis = i (1024).
d-ownership: ACT computes |e1[i,d] - e2[j,d]| for d in [0,20) as an Abs
activation (bias = -e2[j,d]); d in [20,32) goes to DVE as 6 custom
ABSDIFF2_MAX ops, each consuming TWO dims in one 1x pass:
  out = max(|in0 - c0|, |in1 - c1|)
which replaces 2 absdiffs + 1 merge of the stock-op scheme. The remaining
reduction is a wide tt-max tree (2x bf16). Pair ops are emitted two
j-blocks ahead of the merge stage so DVE never waits on ACT's first
block; e2/-e2 are prepped host-side (contiguous [128, 16*32] f32) so both
engines start within a few us. The last block uses narrow sequential
merges that chase ACT's slots to shrink the kernel tail. Output is bf16,
upcast on host.
"""

import sys

if "/opt/trn_rl_repo" not in sys.path:
    sys.path.insert(0, "/opt/trn_rl_repo")

from contextlib import ExitStack

import ml_dtypes
import numpy as np

import concourse.bacc as bacc
import concourse.bass as bass
import concourse.tile as tile
from concourse import mybir

BF16 = ml_dtypes.bfloat16

N = 4096          # rows of embed1 (= rows of embed2)
D = 32            # feature dim
N_CORES = 8
N_IQ = 4          # i split (embed1 rows)
N_JH = 2          # j split (embed2 rows)
I_PER = N // N_IQ       # 1024 per core
J_PER = N // N_JH       # 2048 per core
JB = J_PER // 128       # 16 j-blocks per core
U = I_PER               # free-dim elements per d-slot

# d ownership: ACT does d[0:20] (one Abs per dim), DVE does d[20:32] as
# 6 pair ops (two dims per op).
N_ACT = 20
N_PAIR = (D - N_ACT) // 2
assert N_ACT + 2 * N_PAIR == D
ACT_CHUNKS = (2, 6, 6, 6)
assert sum(ACT_CHUNKS) == N_ACT
PAIR_CHUNKS = (4, 4, 4)        # e1 pair-slab DMA chunks (slots)
assert sum(PAIR_CHUNKS) == 2 * N_PAIR
PAIR_AHEAD = 2                 # pair ops run this many blocks ahead of merges

_nc_cache = None
_pair_op = None


def _register_pair_op():
    """Register the ABSDIFF2_MAX custom DVE op (runtime append to dve_ops.OPS).

    out = max(|in0 - s0|, |in1 - s1|) per element; s0/s1 are per-partition
    f32 scalars. Runs at 1 elem/cycle (1x) but consumes two feature dims
    and performs their pair-max in a single instruction.
    """
    global _pair_op
    if _pair_op is not None:
        return _pair_op
    from concourse.dve_spec import Spec, Src0, Src1, C0, C1, maxx, lower, _has_src1
    from concourse.dve_uop import DveOpSpec
    from concourse.dve_ops import (
        DveOp, OPS, _SUB_OPCODE_FOR_NAME, _CUSTOM_DVE_ROW_BASE,
    )

    name = "ABSDIFF2_MAX_ANT"
    for op in OPS:
        if op.name == name:
            _pair_op = op
            return op

    body = maxx(maxx(Src0 - C0, C0 - Src0), maxx(Src1 - C1, C1 - Src1))
    ref = lambda in0, in1, s0, s1, imm2: np.maximum(
        np.abs(in0 - s0), np.abs(in1 - s1))
    spec = Spec(body=body, reference=ref)
    row = _CUSTOM_DVE_ROW_BASE + len(OPS)
    assert row < 0x20
    shas = {}
    for ver in ("v3", "v4"):
        s = DveOpSpec(name=name, opcode=row, uops=lower(spec, ver=ver),
                      rd1_en=_has_src1(spec))
        shas[ver] = s.sha(ver)
    op = DveOp(name, spec, subdim=False, uops_sha=shas)
    OPS.append(op)
    _SUB_OPCODE_FOR_NAME[name] = row
    _pair_op = op
    return op


def _build_nc():
    pair_op = _register_pair_op()

    nc = bacc.Bacc(
        trn_type="TRN2",
        target_bir_lowering=False,
        debug=False,
        num_devices=N_CORES,
    )

    dt_bf16 = mybir.dt.bfloat16
    dt_f32 = mybir.dt.float32

    # e1 slab transposed to d-major and broadcast across 128 partitions
    # (host side), split by absdiff owner.
    e1r_act = nc.declare_dram_parameter("e1r_act", [128, N_ACT * U], dt_bf16, isOutput=False)
    e1r_pair = nc.declare_dram_parameter("e1r_pair", [128, 2 * N_PAIR * U], dt_bf16, isOutput=False)
    # e2 j-half slab, pre-transposed host-side to [128, JB*D] (p-major) so the
    # load is one contiguous DMA; e2n = -e2 for the ACT bias.
    e2b = nc.declare_dram_parameter("e2b", [128, JB * D], dt_f32, isOutput=False)
    e2nb = nc.declare_dram_parameter("e2nb", [128, JB * D], dt_f32, isOutput=False)
    out = nc.declare_dram_parameter("out", [J_PER, I_PER], dt_bf16, isOutput=True)

    vmax = mybir.AluOpType.max

    with tile.TileContext(nc) as tc, ExitStack() as ctx:
        p_e1 = ctx.enter_context(tc.tile_pool(name="e1", bufs=1))
        p_e2 = ctx.enter_context(tc.tile_pool(name="e2", bufs=1))
        p_act = ctx.enter_context(tc.tile_pool(name="ract", bufs=2))
        p_pair = ctx.enter_context(tc.tile_pool(name="rpair", bufs=PAIR_AHEAD + 1))
        p_out = ctx.enter_context(tc.tile_pool(name="out", bufs=2))

        # --- one-time loads; ordered so DVE (pairs) and ACT start early ---
        t_e2 = p_e2.tile([128, JB * D], dt_f32, tag="e2")
        t_e2n = p_e2.tile([128, JB * D], dt_f32, tag="e2n")
        nc.sync.dma_start(t_e2[:], e2b[:, :])

        t_e1p = p_e1.tile([128, 2 * N_PAIR * U], dt_bf16, tag="e1p")
        t_e1a_chunks = []

        # pair chunk a first (DVE's first pair ops), then e2n + ACT chunk 0,
        # then the rest interleaved smallest-dependency-first
        po = 0
        pair_dmas = []
        for ci, csz in enumerate(PAIR_CHUNKS):
            pair_dmas.append((po, csz))
            po += csz
        nc.sync.dma_start(t_e1p[:, pair_dmas[0][0] * U:(pair_dmas[0][0] + pair_dmas[0][1]) * U],
                          e1r_pair[:, pair_dmas[0][0] * U:(pair_dmas[0][0] + pair_dmas[0][1]) * U])
        nc.sync.dma_start(t_e2n[:], e2nb[:, :])

        off_a = ACT_CHUNKS[0]
        t0 = p_e1.tile([128, ACT_CHUNKS[0] * U], dt_bf16, tag="e1a0")
        nc.sync.dma_start(t0[:], e1r_act[:, :off_a * U])
        t_e1a_chunks.append((0, ACT_CHUNKS[0], t0))

        nc.sync.dma_start(t_e1p[:, pair_dmas[1][0] * U:(pair_dmas[1][0] + pair_dmas[1][1]) * U],
                          e1r_pair[:, pair_dmas[1][0] * U:(pair_dmas[1][0] + pair_dmas[1][1]) * U])

        t1 = p_e1.tile([128, ACT_CHUNKS[1] * U], dt_bf16, tag="e1a1")
        nc.sync.dma_start(t1[:], e1r_act[:, off_a * U:(off_a + ACT_CHUNKS[1]) * U])
        t_e1a_chunks.append((off_a, ACT_CHUNKS[1], t1))
        off_a += ACT_CHUNKS[1]

        nc.sync.dma_start(t_e1p[:, pair_dmas[2][0] * U:(pair_dmas[2][0] + pair_dmas[2][1]) * U],
                          e1r_pair[:, pair_dmas[2][0] * U:(pair_dmas[2][0] + pair_dmas[2][1]) * U])

        for ci, csz in enumerate(ACT_CHUNKS[2:], 2):
            t = p_e1.tile([128, csz * U], dt_bf16, tag=f"e1a{ci}")
            nc.sync.dma_start(t[:], e1r_act[:, off_a * U:(off_a + csz) * U])
            t_e1a_chunks.append((off_a, csz, t))
            off_a += csz

        w = U
        i_lo = 0
        pair_tiles = {}

        def emit_pairs(jb):
            """6 custom pair-absdiff ops for j-block jb (DVE, no ACT dep)."""
            r_p = p_pair.tile([128, N_PAIR * w], dt_bf16, tag="rpair")
            pair_tiles[jb] = r_p
            for k in range(N_PAIR):
                d0 = N_ACT + 2 * k
                d1 = d0 + 1
                nc.vector._custom_dve(
                    pair_op,
                    out=r_p[:, k * w:(k + 1) * w],
                    in0=t_e1p[:, (2 * k) * U + i_lo:(2 * k) * U + i_lo + w],
                    in1=t_e1p[:, (2 * k + 1) * U + i_lo:(2 * k + 1) * U + i_lo + w],
                    s0=t_e2[:, jb * D + d0: jb * D + d0 + 1],
                    s1=t_e2[:, jb * D + d1: jb * D + d1 + 1],
                )

        def emit_act(jb):
            """20 ACT absdiffs for j-block jb."""
            r_a = p_act.tile([128, N_ACT * w], dt_bf16, tag="ract")
            for off, csz, t in t_e1a_chunks:
                for kk in range(csz):
                    k = off + kk
                    d = k
                    # out = Abs(in * 1.0 + (-e2col))
                    nc.scalar.activation(
                        r_a[:, k * w:(k + 1) * w],
                        t[:, kk * U + i_lo:kk * U + i_lo + w],
                        mybir.ActivationFunctionType.Abs,
                        bias=t_e2n[:, jb * D + d: jb * D + d + 1],
                        scale=1.0,
                    )
            return r_a

        def emit_merges(jb, r_a, chase=False):
            r_p = pair_tiles.pop(jb)
            # pair-slot tree: 6 -> 3 -> 2 -> 1
            nc.vector.tensor_tensor(r_p[:, :3 * w], r_p[:, :3 * w], r_p[:, 3 * w:6 * w], op=vmax)
            nc.vector.tensor_tensor(r_p[:, :w], r_p[:, :w], r_p[:, w:2 * w], op=vmax)
            nc.vector.tensor_tensor(r_p[:, :w], r_p[:, :w], r_p[:, 2 * w:3 * w], op=vmax)

            # ACT region: 20 slots -> 10 -> 5 -> (2+ragged) -> 1
            nc.vector.tensor_tensor(r_a[:, :10 * w], r_a[:, :10 * w], r_a[:, 10 * w:20 * w], op=vmax)
            nc.vector.tensor_tensor(r_a[:, :5 * w], r_a[:, :5 * w], r_a[:, 5 * w:10 * w], op=vmax)
            nc.vector.tensor_tensor(r_a[:, :2 * w], r_a[:, :2 * w], r_a[:, 2 * w:4 * w], op=vmax)
            nc.vector.tensor_tensor(r_a[:, :w], r_a[:, :w], r_a[:, w:2 * w], op=vmax)
            nc.vector.tensor_tensor(r_a[:, :w], r_a[:, :w], r_a[:, 4 * w:5 * w], op=vmax)

            # final merge + store
            t_out = p_out.tile([128, w], dt_bf16, tag="out")
            nc.vector.tensor_tensor(t_out[:], r_a[:, :w], r_p[:, :w], op=vmax)
            nc.sync.dma_start(out[jb * 128:(jb + 1) * 128, i_lo:i_lo + w], t_out[:])

        def emit_tail_block(jb):
            """Last block: ACT + chase merges in two 512-wide halves so the
            final output DMA overlaps ACT's last instructions (short tail)."""
            r_a = p_act.tile([128, N_ACT * w], dt_bf16, tag="ract")
            r_p = pair_tiles.pop(jb)
            # full-width pair tree: 6 -> 3 -> 2 -> 1
            nc.vector.tensor_tensor(r_p[:, :3 * w], r_p[:, :3 * w], r_p[:, 3 * w:6 * w], op=vmax)
            nc.vector.tensor_tensor(r_p[:, :w], r_p[:, :w], r_p[:, w:2 * w], op=vmax)
            nc.vector.tensor_tensor(r_p[:, :w], r_p[:, :w], r_p[:, 2 * w:3 * w], op=vmax)
            hw = w // 2
            for h_lo in (0, hw):
                for off, csz, t in t_e1a_chunks:
                    for kk in range(csz):
                        k = off + kk
                        nc.scalar.activation(
                            r_a[:, k * w + h_lo:k * w + h_lo + hw],
                            t[:, kk * U + h_lo:kk * U + h_lo + hw],
                            mybir.ActivationFunctionType.Abs,
                            bias=t_e2n[:, jb * D + k: jb * D + k + 1],
                            scale=1.0,
                        )
                # narrow sequential accumulation chasing ACT's slots
                for k in range(1, N_ACT):
                    nc.vector.tensor_tensor(
                        r_a[:, h_lo:h_lo + hw], r_a[:, h_lo:h_lo + hw],
                        r_a[:, k * w + h_lo:k * w + h_lo + hw], op=vmax)
                t_out = p_out.tile([128, hw], dt_bf16, tag="out")
                nc.vector.tensor_tensor(t_out[:], r_a[:, h_lo:h_lo + hw],
                                        r_p[:, h_lo:h_lo + hw], op=vmax)
                nc.sync.dma_start(out[jb * 128:(jb + 1) * 128, h_lo:h_lo + hw], t_out[:])

        # software pipeline: pairs run PAIR_AHEAD blocks ahead of merges
        for jb in range(PAIR_AHEAD):
            emit_pairs(jb)
        for jb in range(JB):
            if jb + PAIR_AHEAD < JB:
                emit_pairs(jb + PAIR_AHEAD)
            if jb == JB - 1:
                emit_tail_block(jb)
            else:
                r_a = emit_act(jb)
                emit_merges(jb, r_a, chase=False)

    nc.finalize()
    return nc


def _get_nc():
    global _nc_cache
    if _nc_cache is None:
        _nc_cache = _build_nc()
    return _nc_cache


def make_in_maps(embed1: np.ndarray, embed2: np.ndarray):
    """Host-side sharding/prep. Returns in_maps for cores 0..7.

    Core c: iq = c % N_IQ, jh = c // N_IQ.
    """
    embed1 = np.asarray(embed1, dtype=np.float32)
    embed2 = np.asarray(embed2, dtype=np.float32)
    in_maps = []
    for c in range(N_CORES):
        iq, jh = c % N_IQ, c // N_IQ
        e1_slab = embed1[iq * I_PER:(iq + 1) * I_PER, :]      # [1024, 32]
        # d-major flatten, bf16, broadcast to 128 partitions
        flat = np.ascontiguousarray(e1_slab.T).reshape(-1).astype(BF16)  # [32*1024]
        rep = np.ascontiguousarray(np.broadcast_to(flat[None, :], (128, D * I_PER)))
        e2_slab = embed2[jh * J_PER:(jh + 1) * J_PER, :]  # [2048, 32]
        # [128, JB*D] p-major: e2p[p, jb*D+d] = e2_slab[jb*128+p, d]
        e2p = np.ascontiguousarray(
            e2_slab.reshape(JB, 128, D).transpose(1, 0, 2).reshape(128, JB * D))
        in_maps.append({
            "e1r_act": np.ascontiguousarray(rep[:, :N_ACT * U]),
            "e1r_pair": np.ascontiguousarray(rep[:, N_ACT * U:]),
            "e2b": e2p,
            "e2nb": np.ascontiguousarray(-e2p),
        })
    return in_maps


def assemble(results) -> np.ndarray:
    """results: list of per-core dicts with 'out' [J_PER, I_PER] bf16."""
    full = np.empty((N, N), dtype=np.float32)
    for c in range(N_CORES):
        iq, jh = c % N_IQ, c // N_IQ
        blk = np.asarray(results[c]["out"]).astype(np.float32)  # [2048, 1024]
        full[iq * I_PER:(iq + 1) * I_PER, jh * J_PER:(jh + 1) * J_PER] = blk.T
    return full


def kernel(embed1: np.ndarray, embed2: np.ndarray) -> np.ndarray:
    from concourse.bass_utils import run_bass_kernel_spmd

    nc = _get_nc()
    in_maps = make_in_maps(np.asarray(embed1), np.asarray(embed2))
    res = run_bass_kernel_spmd(nc, in_maps, core_ids=list(range(N_CORES)))
    return assemble(res.results)


if __name__ == "__main__":
    e1 = np.random.randn(N, D).astype(np.float32)
    e2 = np.random.randn(N, D).astype(np.float32)
    out = kernel(embed1=e1, embed2=e2)
    ref = np.max(np.abs(e1[:, None, :] - e2[None, :, :]), axis=2)
    err = np.abs(out - ref).max() / np.abs(ref).max()
    print("rel err:", err)
